# revision 11
# baseline (speedup 1.0000x reference)
"""LocationSensitiveAttention Trainium2 kernel.

Full (unsharded) inputs in, full outputs out. Internally: data-parallel over
the batch axis across 8 NeuronCores (16 batch elements per core); all params
replicated.

Per-core device algorithm (T=1024, B_loc=16, H=512, A=128, K=32, F=31):
  pk^T[a, (b,t)]  = sum_h Wk[h,a] * enc[t,b,h]      (PE, Wk stationary, enc^T tiles)
  pl^T[a, (b,t)]  = sum_f U[f,a] * pa_pad[b, t+f]   (PE, U = filt^T @ W_loc, im2col via
                                                     shifted-replica sliding-window AP)
  S = tanh(pk+pl+pq[b])                             (ACT, pq as per-partition bias)
  score = v . S                                     (PE, tanh-tile stationary, v moving)
  alignment = exp(score)/Z                          (per-b softmax, ones-matmul sums)
  context[b,:] = sum_t alignment * enc              (PE, alignment column stationary)

The whole b-loop is software-pipelined by the Tile scheduler; big loads are
issued from GpSimd (SWDGE) so the Sync sequencer doesn't serialize transfers.
"""

import numpy as np
import ml_dtypes

import concourse.bass as bass
import concourse.tile as tile
from concourse import bacc, mybir
from concourse.bass_utils import run_bass_kernel_spmd

BF16 = mybir.dt.bfloat16
F32 = mybir.dt.float32
AF = mybir.ActivationFunctionType

T, B, H = 1024, 128, 512
A, K, F = 128, 32, 31
NCORES = 8
BL = B // NCORES          # 16 batch elements per core
PAD = (F - 1) // 2        # 15
ROW = T + 2 * PAD         # 1054: padded per-batch alignment row
PA_LEN = BL * ROW + 32    # shifted-window source length
TH = T // 512             # 2 t-halves per batch element
NB = 512 // 128           # 4 128-blocks per t-half
HK = H // 128             # 4 contraction chunks


def _build_program() -> bacc.Bacc:
    nc = bacc.Bacc("TRN2", target_bir_lowering=False, debug=False)

    encT = nc.dram_tensor("encT", [BL, H, T], BF16, kind="ExternalInput").ap()
    encN = nc.dram_tensor("encN", [BL, T, H], BF16, kind="ExternalInput").ap()
    paT = nc.dram_tensor("paT", [PA_LEN], BF16, kind="ExternalInput").ap()
    qT = nc.dram_tensor("qT", [H, BL], F32, kind="ExternalInput").ap()
    wk = nc.dram_tensor("wk", [H, A], BF16, kind="ExternalInput").ap()
    wq = nc.dram_tensor("wq", [H, A], F32, kind="ExternalInput").ap()
    wloc = nc.dram_tensor("wloc", [K, A], F32, kind="ExternalInput").ap()
    filt = nc.dram_tensor("filt", [K, F], F32, kind="ExternalInput").ap()
    vv = nc.dram_tensor("vv", [A, 1], BF16, kind="ExternalInput").ap()
    ctx_out = nc.dram_tensor("ctx_out", [BL, H], F32, kind="ExternalOutput").ap()
    al_out = nc.dram_tensor("al_out", [T, BL], F32, kind="ExternalOutput").ap()

    with tile.TileContext(nc) as tc:
        from contextlib import ExitStack

        with ExitStack() as ctx:
            cp = ctx.enter_context(tc.tile_pool(name="consts", bufs=1))

            # --- replicated params into SBUF ---
            wk_sb = cp.tile([128, HK, A], BF16)
            nc.sync.dma_start(wk_sb[:], wk.rearrange("(hk p) a -> p hk a", p=128))
            wq_sb = cp.tile([128, HK, A], F32)
            nc.sync.dma_start(wq_sb[:], wq.rearrange("(hk p) a -> p hk a", p=128))
            qT_sb = cp.tile([128, HK, BL], F32)
            nc.sync.dma_start(qT_sb[:], qT.rearrange("(hk p) b -> p hk b", p=128))
            filt_sb = cp.tile([K, F], F32)
            nc.sync.dma_start(filt_sb[:], filt)
            wloc_sb = cp.tile([K, A], F32)
            nc.sync.dma_start(wloc_sb[:], wloc)
            vv_sb = cp.tile([A, 1], BF16)
            nc.sync.dma_start(vv_sb[:], vv)
            # padded previous_alignment, replicated with +1-element shift per
            # partition: pa_rep[f, x] = paT[x + f] -> conv windows are plain
            # [31, 512] slices.
            pa_rep = cp.tile([F, BL * ROW], BF16)
            pa_src = bass.AP(paT.tensor, 0, [[1, F], [1, BL * ROW]])
            nc.gpsimd.dma_start(pa_rep[:], pa_src)

            ones_col = cp.tile([128, 1], F32)
            nc.any.memset(ones_col[:], 1.0)
            ones_row = cp.tile([1, 128], F32)
            nc.any.memset(ones_row[:], 1.0)

            # --- tiny setup matmuls: U = filt^T @ wloc, pq^T = wq^T @ q^T ---
            with tc.tile_pool(name="setup_ps", bufs=1, space="PSUM") as sps:
                u_ps = sps.tile([F, A], F32)
                nc.tensor.matmul(u_ps[:], filt_sb[:], wloc_sb[:], start=True, stop=True)
                u_sb = cp.tile([F, A], BF16)
                nc.scalar.copy(u_sb[:], u_ps[:])

                pq_ps = sps.tile([A, BL], F32)
                for hk in range(HK):
                    nc.tensor.matmul(
                        pq_ps[:], wq_sb[:, hk], qT_sb[:, hk],
                        start=(hk == 0), stop=(hk == HK - 1),
                    )
                pq_sb = cp.tile([A, BL], F32)
                nc.scalar.copy(pq_sb[:], pq_ps[:])

            # --- fused per-b pipeline: scores -> softmax(b) -> context(b) ---
            io_pool = ctx.enter_context(tc.tile_pool(name="io", bufs=3))
            wrk = ctx.enter_context(tc.tile_pool(name="wrk", bufs=3))
            ps_s = ctx.enter_context(tc.tile_pool(name="ps_s", bufs=2, space="PSUM"))
            ps_sc = ctx.enter_context(tc.tile_pool(name="ps_sc", bufs=2, space="PSUM"))
            ps_z = ctx.enter_context(tc.tile_pool(name="ps_z", bufs=1, space="PSUM"))
            ps_c = ctx.enter_context(tc.tile_pool(name="ps_c", bufs=2, space="PSUM"))

            al_f32 = cp.tile([128, TH * NB * BL], F32)   # [p, (b tc)]
            ctx_sb = cp.tile([1, BL * H], F32)

            for b in range(BL):
                # pass 1: scores for all t of this b
                enct = io_pool.tile([128, HK, T], BF16, tag="enct")
                nc.gpsimd.dma_start(
                    enct[:], encT[b].rearrange("(hk p) t -> p hk t", p=128)
                )
                score_ps = ps_sc.tile([128, TH * NB], F32, tag="score")
                for th in range(TH):
                    t0 = th * 512
                    s_ps = ps_s.tile([A, 512], F32, tag="s")
                    for hk in range(HK):
                        nc.tensor.matmul(
                            s_ps[:], wk_sb[:, hk], enct[:, hk, t0:t0 + 512],
                            start=(hk == 0), stop=False,
                        )
                    off = b * ROW + t0
                    nc.tensor.matmul(
                        s_ps[:], u_sb[:], pa_rep[:, off:off + 512],
                        start=False, stop=True,
                    )
                    tanh_t = wrk.tile([A, 512], BF16, tag="tanh")
                    nc.scalar.activation(
                        tanh_t[:], s_ps[:], AF.Tanh, bias=pq_sb[:, b:b + 1]
                    )
                    for c in range(NB):
                        nc.tensor.matmul(
                            score_ps[:, th * NB + c:th * NB + c + 1],
                            tanh_t[:, c * 128:(c + 1) * 128],
                            vv_sb[:],
                            start=True, stop=True,
                        )

                # per-b softmax (scores are O(1): exp without max-subtract)
                expsc = wrk.tile([128, TH * NB], F32, tag="expsc")
                nc.scalar.activation(expsc[:], score_ps[:], AF.Exp)
                zcol_ps = ps_z.tile([1, TH * NB], F32, tag="zcol")
                nc.tensor.matmul(
                    zcol_ps[:], ones_col[:], expsc[:], start=True, stop=True
                )
                zsum = wrk.tile([1, 1], F32, tag="zsum")
                nc.vector.reduce_sum(zsum[:], zcol_ps[:], axis=mybir.AxisListType.X)
                rz = wrk.tile([1, 1], F32, tag="rz")
                nc.vector.reciprocal(rz[:], zsum[:])
                rz128_ps = ps_z.tile([128, 1], F32, tag="rz128")
                nc.tensor.matmul(rz128_ps[:], ones_row[:], rz[:], start=True, stop=True)
                al_b = al_f32[:, b * TH * NB:(b + 1) * TH * NB]
                nc.vector.tensor_scalar_mul(al_b, expsc[:], rz128_ps[:])
                al_bf = wrk.tile([128, TH * NB], BF16, tag="al_bf")
                nc.vector.tensor_copy(al_bf[:], al_b)

                # pass 2: context for this b
                encn = io_pool.tile([128, TH * NB, H], BF16, tag="encn")
                nc.gpsimd.dma_start(
                    encn[:], encN[b].rearrange("(k p) h -> p k h", p=128)
                )
                c_ps = ps_c.tile([1, H], F32, tag="ctx")
                for k in range(TH * NB):
                    nc.tensor.matmul(
                        c_ps[:], al_bf[:, k:k + 1], encn[:, k],
                        start=(k == 0), stop=(k == TH * NB - 1),
                    )
                nc.scalar.copy(ctx_sb[:, b * H:(b + 1) * H], c_ps[:])

            # outputs (DMA APs limited to 3 dims -> one store per token chunk)
            al_src = al_f32[:].rearrange("p (b tc) -> tc p b", tc=TH * NB)
            al_dst = al_out.rearrange("(tc p) b -> tc p b", p=128)
            for tc_i in range(TH * NB):
                nc.sync.dma_start(al_dst[tc_i], al_src[tc_i])
            nc.sync.dma_start(ctx_out.rearrange("b h -> (b h)").unsqueeze(0), ctx_sb[:])

    nc.compile()
    return nc


_PROGRAM_CACHE: list = []


def _get_program() -> bacc.Bacc:
    if not _PROGRAM_CACHE:
        _PROGRAM_CACHE.append(_build_program())
    return _PROGRAM_CACHE[0]


def _host_shard(encoded_tokens, query, previous_alignment, conv_filter, W_loc, W_q, W_k, v):
    bf16 = ml_dtypes.bfloat16
    enc_bt = np.ascontiguousarray(encoded_tokens.transpose(1, 0, 2))  # [B, T, H]
    encN_all = enc_bt.astype(bf16)                                    # [B, T, H]
    encT_all = np.ascontiguousarray(enc_bt.transpose(0, 2, 1)).astype(bf16)  # [B, H, T]

    wk_h = W_k.astype(bf16)
    v_h = v.reshape(A, 1).astype(bf16)
    filt_h = np.ascontiguousarray(conv_filter.reshape(K, F)).astype(np.float32)
    wloc_h = W_loc.astype(np.float32)
    wq_h = W_q.astype(np.float32)

    in_maps = []
    for c in range(NCORES):
        b0 = c * BL
        pa_pad = np.zeros((BL, ROW), np.float32)
        pa_pad[:, PAD:PAD + T] = previous_alignment[:, b0:b0 + BL].T
        pa_flat = np.zeros(PA_LEN, np.float32)
        pa_flat[:BL * ROW] = pa_pad.reshape(-1)
        in_maps.append({
            "encT": encT_all[b0:b0 + BL],
            "encN": encN_all[b0:b0 + BL],
            "paT": pa_flat.astype(bf16),
            "qT": np.ascontiguousarray(query[b0:b0 + BL].T).astype(np.float32),
            "wk": wk_h,
            "wq": wq_h,
            "wloc": wloc_h,
            "filt": filt_h,
            "vv": v_h,
        })
    return in_maps


def run(inputs: dict, trace: bool = False):
    nc = _get_program()
    in_maps = _host_shard(**inputs)
    res = run_bass_kernel_spmd(nc, in_maps, list(range(NCORES)), trace=trace)
    context = np.concatenate([res.results[c]["ctx_out"] for c in range(NCORES)], axis=0)
    alignment = np.concatenate(
        [res.results[c]["al_out"] for c in range(NCORES)], axis=1
    )
    return (context.astype(np.float32), alignment.astype(np.float32)), res


def kernel(**inputs):
    (context, alignment), _ = run(inputs, trace=False)
    return context, alignment


# revision 14
# speedup vs baseline: 1.2365x; 1.2365x over previous
"""LocationSensitiveAttention Trainium2 kernel.

Full (unsharded) inputs in, full outputs out. Internally: data-parallel over
the batch axis across 8 NeuronCores (16 batch elements per core); all params
replicated.

Per-core device algorithm (T=1024, B_loc=16, H=512, A=128, K=32, F=31):
  pk^T[a, (b,t)]  = sum_h Wk[h,a] * enc[t,b,h]      (PE, Wk stationary, enc^T tiles)
  pl^T[a, (b,t)]  = sum_f U[f,a] * pa_pad[b, t+f]   (PE, U = filt^T @ W_loc, im2col via
                                                     shifted-replica sliding-window AP)
  S = tanh(pk+pl+pq[b])                             (ACT, pq as per-partition bias)
  score = v . S                                     (PE, tanh-tile stationary, v moving)
  alignment = exp(score)/Z                          (per-b softmax, ones-matmul sums)
  context[b,:] = sum_t alignment * enc              (PE, alignment column stationary)

The b-loop is explicitly software-pipelined with a 2-stage skew — scores(b),
softmax(b-1), context(b-2) — so the in-order engines never stall on the
per-b serial dependency chain. Big loads are issued from GpSimd (SWDGE) so
the Sync sequencer doesn't serialize transfers.
"""

import numpy as np
import ml_dtypes

import concourse.bass as bass
import concourse.tile as tile
from concourse import bacc, mybir
from concourse.bass_utils import run_bass_kernel_spmd

BF16 = mybir.dt.bfloat16
F32 = mybir.dt.float32
AF = mybir.ActivationFunctionType

T, B, H = 1024, 128, 512
A, K, F = 128, 32, 31
NCORES = 8
BL = B // NCORES          # 16 batch elements per core
PAD = (F - 1) // 2        # 15
ROW = T + 2 * PAD         # 1054: padded per-batch alignment row
PA_LEN = BL * ROW + 32    # shifted-window source length
TH = T // 512             # 2 t-halves per batch element
NB = 512 // 128           # 4 128-blocks per t-half
NC_ = TH * NB             # 8 token chunks per batch element
HK = H // 128             # 4 contraction chunks


def _build_program() -> bacc.Bacc:
    nc = bacc.Bacc("TRN2", target_bir_lowering=False, debug=False)

    encT = nc.dram_tensor("encT", [BL, H, T], BF16, kind="ExternalInput").ap()
    encN = nc.dram_tensor("encN", [BL, T, H], BF16, kind="ExternalInput").ap()
    paT = nc.dram_tensor("paT", [PA_LEN], BF16, kind="ExternalInput").ap()
    # params pre-laid-out host-side as contiguous [128, x] images
    qT = nc.dram_tensor("qT", [128, HK * BL], F32, kind="ExternalInput").ap()
    wk = nc.dram_tensor("wk", [128, HK * A], BF16, kind="ExternalInput").ap()
    wq = nc.dram_tensor("wq", [128, HK * A], F32, kind="ExternalInput").ap()
    wloc = nc.dram_tensor("wloc", [K, A], F32, kind="ExternalInput").ap()
    filt = nc.dram_tensor("filt", [K, F], F32, kind="ExternalInput").ap()
    vv = nc.dram_tensor("vv", [A, 1], BF16, kind="ExternalInput").ap()
    ctx_out = nc.dram_tensor("ctx_out", [BL, H], F32, kind="ExternalOutput").ap()
    al_out = nc.dram_tensor("al_out", [BL, T], F32, kind="ExternalOutput").ap()

    with tile.TileContext(nc) as tc:
        from contextlib import ExitStack

        with ExitStack() as ctx:
            cp = ctx.enter_context(tc.tile_pool(name="consts", bufs=1))
            io_pool = ctx.enter_context(tc.tile_pool(name="io", bufs=3))
            wrk = ctx.enter_context(tc.tile_pool(name="wrk", bufs=3))
            ps_s = ctx.enter_context(tc.tile_pool(name="ps_s", bufs=2, space="PSUM"))
            ps_sc = ctx.enter_context(tc.tile_pool(name="ps_sc", bufs=2, space="PSUM"))
            ps_z = ctx.enter_context(tc.tile_pool(name="ps_z", bufs=1, space="PSUM"))

            # prefetch the first batch element before anything else
            enct_tiles = {}
            encn_tiles = {}

            def load_enct(b):
                t = io_pool.tile([128, HK, T], BF16, tag="enct")
                nc.gpsimd.dma_start(
                    t[:], encT[b].rearrange("(hk p) t -> p hk t", p=128)
                )
                enct_tiles[b] = t

            def load_encn(b):
                t = io_pool.tile([128, NC_, H], BF16, tag="encn")
                nc.gpsimd.dma_start(
                    t[:], encN[b].rearrange("(k p) h -> p k h", p=128)
                )
                encn_tiles[b] = t

            load_enct(0)
            load_encn(0)
            load_enct(1)

            # --- replicated params (contiguous layouts -> cheap DMAs) ---
            wk_sb = cp.tile([128, HK, A], BF16)
            nc.sync.dma_start(wk_sb[:], wk.rearrange("p (hk a) -> p hk a", hk=HK))
            wq_sb = cp.tile([128, HK, A], F32)
            nc.sync.dma_start(wq_sb[:], wq.rearrange("p (hk a) -> p hk a", hk=HK))
            qT_sb = cp.tile([128, HK, BL], F32)
            nc.sync.dma_start(qT_sb[:], qT.rearrange("p (hk b) -> p hk b", hk=HK))
            filt_sb = cp.tile([K, F], F32)
            nc.sync.dma_start(filt_sb[:], filt)
            wloc_sb = cp.tile([K, A], F32)
            nc.sync.dma_start(wloc_sb[:], wloc)
            vv_sb = cp.tile([A, 1], BF16)
            nc.sync.dma_start(vv_sb[:], vv)
            # padded previous_alignment, replicated with +1-element shift per
            # partition: pa_rep[f, x] = paT[x + f] -> conv windows are plain
            # [31, 512] slices.
            pa_rep = cp.tile([F, BL * ROW], BF16)
            pa_src = bass.AP(paT.tensor, 0, [[1, F], [1, BL * ROW]])
            nc.gpsimd.dma_start(pa_rep[:], pa_src)

            ones_col = cp.tile([128, 1], F32)
            nc.any.memset(ones_col[:], 1.0)
            ones_row = cp.tile([1, 128], F32)
            nc.any.memset(ones_row[:], 1.0)
            ident = cp.tile([128, 128], F32)
            from concourse.masks import make_identity
            make_identity(nc, ident[:])

            # --- tiny setup matmuls: U = filt^T @ wloc, pq^T = wq^T @ q^T ---
            with tc.tile_pool(name="setup_ps", bufs=1, space="PSUM") as sps:
                u_ps = sps.tile([F, A], F32)
                nc.tensor.matmul(u_ps[:], filt_sb[:], wloc_sb[:], start=True, stop=True)
                u_sb = cp.tile([F, A], BF16)
                nc.scalar.copy(u_sb[:], u_ps[:])

                pq_ps = sps.tile([A, BL], F32)
                for hk in range(HK):
                    nc.tensor.matmul(
                        pq_ps[:], wq_sb[:, hk], qT_sb[:, hk],
                        start=(hk == 0), stop=(hk == HK - 1),
                    )
                pq_sb = cp.tile([A, BL], F32)
                nc.scalar.copy(pq_sb[:], pq_ps[:])

            al_f32 = cp.tile([128, NC_ * BL], F32)   # [p, (b tc)]
            ctx_sb = cp.tile([1, BL * H], F32)

            score_tiles = {}
            albf_tiles = {}
            zr_tiles = {}
            expsc_tiles = {}

            def stage_scores(b):
                """pass 1: pk+pl matmuls, tanh(+pq bias), v-contraction."""
                enct = enct_tiles[b]
                score_ps = ps_sc.tile([128, NC_], F32, tag="score")
                score_tiles[b] = score_ps
                for th in range(TH):
                    t0 = th * 512
                    s_ps = ps_s.tile([A, 512], F32, tag="s")
                    for hk in range(HK):
                        nc.tensor.matmul(
                            s_ps[:], wk_sb[:, hk], enct[:, hk, t0:t0 + 512],
                            start=(hk == 0), stop=False,
                        )
                    off = b * ROW + t0
                    nc.tensor.matmul(
                        s_ps[:], u_sb[:], pa_rep[:, off:off + 512],
                        start=False, stop=True,
                    )
                    tanh_t = wrk.tile([A, 512], BF16, tag="tanh")
                    nc.scalar.activation(
                        tanh_t[:], s_ps[:], AF.Tanh, bias=pq_sb[:, b:b + 1]
                    )
                    for c in range(NB):
                        nc.tensor.matmul(
                            score_ps[:, th * NB + c:th * NB + c + 1],
                            tanh_t[:, c * 128:(c + 1) * 128],
                            vv_sb[:],
                            start=True, stop=True,
                        )

            def stage_softmax_a(b):
                """exp + Z column-sums (PE part right after v-matmuls of b+1)."""
                expsc = wrk.tile([128, NC_], F32, tag="expsc")
                expsc_tiles[b] = expsc
                nc.scalar.activation(expsc[:], score_tiles[b][:], AF.Exp)
                zr_ps = ps_z.tile([128, 16], F32, tag="zr")
                zr_tiles[b] = zr_ps
                nc.tensor.matmul(
                    zr_ps[0:1, 0:NC_], ones_col[:], expsc[:], start=True, stop=True
                )
                del score_tiles[b]

            def stage_softmax_b(b):
                """reciprocal + broadcast + alignment scale (DVE/PE-tiny)."""
                zr_ps = zr_tiles.pop(b)
                expsc = expsc_tiles.pop(b)
                zsum = wrk.tile([1, 1], F32, tag="zsum")
                nc.vector.reduce_sum(
                    zsum[:], zr_ps[0:1, 0:NC_], axis=mybir.AxisListType.X
                )
                rz = wrk.tile([1, 1], F32, tag="rz")
                nc.vector.reciprocal(rz[:], zsum[:])
                nc.tensor.matmul(
                    zr_ps[:, 8:9], ones_row[:], rz[:], start=True, stop=True
                )
                al_b = al_f32[:, b * NC_:(b + 1) * NC_]
                nc.vector.tensor_scalar_mul(al_b, expsc[:], zr_ps[:, 8:9])
                al_bf = wrk.tile([128, NC_], BF16, tag="al_bf")
                nc.vector.tensor_copy(al_bf[:], al_b)
                albf_tiles[b] = al_bf

            def stage_context(b, ps_c):
                al_bf = albf_tiles.pop(b)
                encn = encn_tiles.pop(b)
                c_ps = ps_c.tile([1, H], F32, tag="ctx")
                for k in range(NC_):
                    nc.tensor.matmul(
                        c_ps[:], al_bf[:, k:k + 1], encn[:, k],
                        start=(k == 0), stop=(k == NC_ - 1),
                    )
                nc.scalar.copy(ctx_sb[:, b * H:(b + 1) * H], c_ps[:])

            with tc.tile_pool(name="ps_c", bufs=2, space="PSUM") as ps_c:
                for i in range(BL + 2):
                    if i < BL:
                        if i + 1 < BL:
                            load_encn(i + 1)
                        if i + 2 < BL:
                            load_enct(i + 2)
                        stage_scores(i)
                    if 1 <= i <= BL:
                        stage_softmax_a(i - 1)
                    if 2 <= i <= BL + 1:
                        stage_context(i - 2, ps_c)
                    if 1 <= i <= BL:
                        stage_softmax_b(i - 1)

            # --- outputs ---
            # transpose alignment on PE so the store is one contiguous DMA:
            # al_f32 [p, (b tc)] -> al_t [(b tc), p]; then partition (b*8+tc)
            # holds t-run [tc*128 .. tc*128+128) of batch b -> flat [BL*T].
            with tc.tile_pool(name="ps_t", bufs=1, space="PSUM") as ps_t:
                al_tp = ps_t.tile([128, 128], F32)
                nc.tensor.transpose(al_tp[:], al_f32[:], ident[:])
                al_t = cp.tile([128, 128], F32)
                nc.vector.tensor_copy(al_t[:], al_tp[:])
            nc.sync.dma_start(
                al_out.rearrange("b t -> (b t)").rearrange("(q p) -> q p", p=128),
                al_t[:],
            )
            nc.sync.dma_start(ctx_out.rearrange("b h -> (b h)").unsqueeze(0), ctx_sb[:])

    nc.compile()
    return nc


_PROGRAM_CACHE: list = []


def _get_program() -> bacc.Bacc:
    if not _PROGRAM_CACHE:
        _PROGRAM_CACHE.append(_build_program())
    return _PROGRAM_CACHE[0]


def _host_shard(encoded_tokens, query, previous_alignment, conv_filter, W_loc, W_q, W_k, v):
    bf16 = ml_dtypes.bfloat16
    enc_bt = np.ascontiguousarray(encoded_tokens.transpose(1, 0, 2))  # [B, T, H]
    encN_all = enc_bt.astype(bf16)                                    # [B, T, H]
    encT_all = np.ascontiguousarray(enc_bt.transpose(0, 2, 1)).astype(bf16)  # [B, H, T]

    # device layout [p, (hk x)]: row p holds W[hk*128+p, :] for hk = 0..3
    def chunked(w, dt):
        return np.ascontiguousarray(
            w.reshape(HK, 128, -1).transpose(1, 0, 2).reshape(128, -1)
        ).astype(dt)

    wk_h = chunked(W_k, bf16)
    wq_h = chunked(W_q, np.float32)
    v_h = v.reshape(A, 1).astype(bf16)
    filt_h = np.ascontiguousarray(conv_filter.reshape(K, F)).astype(np.float32)
    wloc_h = W_loc.astype(np.float32)

    in_maps = []
    for c in range(NCORES):
        b0 = c * BL
        pa_pad = np.zeros((BL, ROW), np.float32)
        pa_pad[:, PAD:PAD + T] = previous_alignment[:, b0:b0 + BL].T
        pa_flat = np.zeros(PA_LEN, np.float32)
        pa_flat[:BL * ROW] = pa_pad.reshape(-1)
        in_maps.append({
            "encT": encT_all[b0:b0 + BL],
            "encN": encN_all[b0:b0 + BL],
            "paT": pa_flat.astype(bf16),
            "qT": chunked(query[b0:b0 + BL].T, np.float32),
            "wk": wk_h,
            "wq": wq_h,
            "wloc": wloc_h,
            "filt": filt_h,
            "vv": v_h,
        })
    return in_maps


def run(inputs: dict, trace: bool = False):
    nc = _get_program()
    in_maps = _host_shard(**inputs)
    res = run_bass_kernel_spmd(nc, in_maps, list(range(NCORES)), trace=trace)
    context = np.concatenate([res.results[c]["ctx_out"] for c in range(NCORES)], axis=0)
    # al_out is [BL, T] per core -> [B, T] -> transpose to [T, B]
    al_bt = np.concatenate([res.results[c]["al_out"] for c in range(NCORES)], axis=0)
    alignment = np.ascontiguousarray(al_bt.T)
    return (context.astype(np.float32), alignment.astype(np.float32)), res


def kernel(**inputs):
    (context, alignment), _ = run(inputs, trace=False)
    return context, alignment


# revision 19
# speedup vs baseline: 1.8702x; 1.5125x over previous
"""LocationSensitiveAttention Trainium2 kernel.

Full (unsharded) inputs in, full outputs out. Internally: data-parallel over
the batch axis across 8 NeuronCores (16 batch elements per core); all params
replicated.

Per-core device algorithm (T=1024, B_loc=16, H=512, A=128, K=32, F=31):
  pk^T[a, (b,t)]  = sum_h Wk[h,a] * enc[t,b,h]      (PE, Wk stationary, enc^T tiles)
  pl^T[a, (b,t)]  = sum_f U[f,a] * pa_pad[b, t+f]   (PE, U = filt^T @ W_loc, im2col via
                                                     shifted-replica sliding-window AP)
  S = tanh(pk+pl+pq[b])                             (ACT, pq as per-partition bias)
  score = v . S                                     (PE, tanh-tile stationary, v moving)
  alignment = exp(score)/Z                          (per-b softmax, ones-matmul sums)
  context[b,:] = sum_t alignment * enc              (PE, alignment column stationary)

The b-loop is explicitly software-pipelined with a 2-stage skew — scores(b),
softmax(b-1), context(b-2) — so the in-order engines never stall on the
per-b serial dependency chain. Big loads are issued from GpSimd (SWDGE) so
the Sync sequencer doesn't serialize transfers.
"""

import numpy as np
import ml_dtypes

import concourse.bass as bass
import concourse.tile as tile
from concourse import bacc, mybir
from concourse.bass_utils import run_bass_kernel_spmd

BF16 = mybir.dt.bfloat16
F32 = mybir.dt.float32
AF = mybir.ActivationFunctionType

T, B, H = 1024, 128, 512
A, K, F = 128, 32, 31
NCORES = 8
BL = B // NCORES          # 16 batch elements per core
PAD = (F - 1) // 2        # 15
ROW = T + 2 * PAD         # 1054: padded per-batch alignment row
PA_LEN = BL * ROW + 32    # shifted-window source length
TH = T // 512             # 2 t-halves per batch element
NB = 512 // 128           # 4 128-blocks per t-half
NC_ = TH * NB             # 8 token chunks per batch element
HK = H // 128             # 4 contraction chunks


def _build_program() -> bacc.Bacc:
    nc = bacc.Bacc("TRN2", target_bir_lowering=False, debug=False)

    encT = nc.dram_tensor("encT", [BL, H, T], BF16, kind="ExternalInput").ap()
    encN = nc.dram_tensor("encN", [BL, T, H], BF16, kind="ExternalInput").ap()
    paT = nc.dram_tensor("paT", [PA_LEN], BF16, kind="ExternalInput").ap()
    # params pre-laid-out host-side as contiguous [128, x] images
    qT = nc.dram_tensor("qT", [128, HK * BL], F32, kind="ExternalInput").ap()
    wk = nc.dram_tensor("wk", [128, HK * A], BF16, kind="ExternalInput").ap()
    wq = nc.dram_tensor("wq", [128, HK * A], F32, kind="ExternalInput").ap()
    wloc = nc.dram_tensor("wloc", [K, A], F32, kind="ExternalInput").ap()
    filt = nc.dram_tensor("filt", [K, F], F32, kind="ExternalInput").ap()
    vv = nc.dram_tensor("vv", [A, 1], BF16, kind="ExternalInput").ap()
    ctx_out = nc.dram_tensor("ctx_out", [BL, H], F32, kind="ExternalOutput").ap()
    al_out = nc.dram_tensor("al_out", [BL, T], F32, kind="ExternalOutput").ap()

    with tile.TileContext(nc) as tc:
        from contextlib import ExitStack

        with ExitStack() as ctx:
            cp = ctx.enter_context(tc.tile_pool(name="consts", bufs=1))
            io_pool = ctx.enter_context(tc.tile_pool(name="io", bufs=3))
            wrk = ctx.enter_context(tc.tile_pool(name="wrk", bufs=3))
            wrk5 = ctx.enter_context(tc.tile_pool(name="wrk5", bufs=5))
            ps_s = ctx.enter_context(tc.tile_pool(name="ps_s", bufs=2, space="PSUM"))
            ps_sc = ctx.enter_context(tc.tile_pool(name="ps_sc", bufs=2, space="PSUM"))
            ps_z = ctx.enter_context(tc.tile_pool(name="ps_z", bufs=1, space="PSUM"))

            # prefetch the first batch element before anything else
            enct_tiles = {}
            encn_tiles = {}

            def load_enct(b):
                t = io_pool.tile([128, HK, T], BF16, tag="enct")
                nc.gpsimd.dma_start(
                    t[:], encT[b].rearrange("(hk p) t -> p hk t", p=128)
                )
                enct_tiles[b] = t

            def load_encn(b):
                t = io_pool.tile([128, NC_, H], BF16, tag="encn")
                nc.gpsimd.dma_start(
                    t[:], encN[b].rearrange("(k p) h -> p k h", p=128)
                )
                encn_tiles[b] = t

            load_enct(0)
            load_encn(0)
            load_enct(1)

            # --- replicated params (contiguous layouts -> cheap DMAs) ---
            wk_sb = cp.tile([128, HK, A], BF16)
            nc.sync.dma_start(wk_sb[:], wk.rearrange("p (hk a) -> p hk a", hk=HK))
            wq_sb = cp.tile([128, HK, A], F32)
            nc.sync.dma_start(wq_sb[:], wq.rearrange("p (hk a) -> p hk a", hk=HK))
            qT_sb = cp.tile([128, HK, BL], F32)
            nc.sync.dma_start(qT_sb[:], qT.rearrange("p (hk b) -> p hk b", hk=HK))
            filt_sb = cp.tile([K, F], F32)
            nc.sync.dma_start(filt_sb[:], filt)
            wloc_sb = cp.tile([K, A], F32)
            nc.sync.dma_start(wloc_sb[:], wloc)
            vv_sb = cp.tile([A, 1], BF16)
            nc.sync.dma_start(vv_sb[:], vv)
            # padded previous_alignment, replicated with +1-element shift per
            # partition: pa_b[f, x] = paT[b*ROW + x + f] -> conv windows are
            # plain [31, 512] slices. Loaded per-b (31-partition DMAs land on
            # few SDMA engines; keep them small + off the gpsimd queue).
            pa_tiles = {}
            for b in range(BL):
                pa_b = cp.tile([F, ROW], BF16, tag=f"pa{b}")
                nc.sync.dma_start(
                    pa_b[:], bass.AP(paT.tensor, b * ROW, [[1, F], [1, ROW]])
                )
                pa_tiles[b] = pa_b

            ones_col = cp.tile([128, 1], F32)
            nc.any.memset(ones_col[:], 1.0)
            ones_row = cp.tile([1, 128], F32)
            nc.any.memset(ones_row[:], 1.0)
            ident = cp.tile([128, 128], F32)
            from concourse.masks import make_identity
            make_identity(nc, ident[:])

            # --- tiny setup matmuls: U = filt^T @ wloc, pq^T = wq^T @ q^T ---
            with tc.tile_pool(name="setup_ps", bufs=1, space="PSUM") as sps:
                u_ps = sps.tile([F, A], F32)
                nc.tensor.matmul(u_ps[:], filt_sb[:], wloc_sb[:], start=True, stop=True)
                u_sb = cp.tile([F, A], BF16)
                nc.scalar.copy(u_sb[:], u_ps[:])

                pq_ps = sps.tile([A, BL], F32)
                for hk in range(HK):
                    nc.tensor.matmul(
                        pq_ps[:], wq_sb[:, hk], qT_sb[:, hk],
                        start=(hk == 0), stop=(hk == HK - 1),
                    )
                pq_sb = cp.tile([A, BL], F32)
                nc.scalar.copy(pq_sb[:], pq_ps[:])

            al_f32 = cp.tile([128, NC_ * BL], F32)   # [p, (b tc)]
            ctx_sb = cp.tile([1, BL * H], F32)

            score_tiles = {}
            albf_tiles = {}
            zr_tiles = {}
            expsc_tiles = {}

            tanh_tiles = {}

            def stage_s(b):
                """pk+pl matmuls + tanh(+pq bias)."""
                enct = enct_tiles.pop(b)
                for th in range(TH):
                    t0 = th * 512
                    s_ps = ps_s.tile([A, 512], F32, tag="s")
                    for hk in range(HK):
                        nc.tensor.matmul(
                            s_ps[:], wk_sb[:, hk], enct[:, hk, t0:t0 + 512],
                            start=(hk == 0), stop=False,
                        )
                    nc.tensor.matmul(
                        s_ps[:], u_sb[:], pa_tiles[b][:, t0:t0 + 512],
                        start=False, stop=True,
                    )
                    tanh_t = wrk5.tile([A, 512], BF16, tag="tanh")
                    nc.scalar.activation(
                        tanh_t[:], s_ps[:], AF.Tanh, bias=pq_sb[:, b:b + 1]
                    )
                    tanh_tiles[(b, th)] = tanh_t

            def stage_v(b):
                """score = v . tanhS (tanh tiles stationary)."""
                score_ps = ps_sc.tile([128, NC_], F32, tag="score")
                score_tiles[b] = score_ps
                for th in range(TH):
                    tanh_t = tanh_tiles.pop((b, th))
                    for c in range(NB):
                        nc.tensor.matmul(
                            score_ps[:, th * NB + c:th * NB + c + 1],
                            tanh_t[:, c * 128:(c + 1) * 128],
                            vv_sb[:],
                            start=True, stop=True,
                        )

            def stage_exp(b):
                expsc = wrk.tile([128, NC_], F32, tag="expsc")
                expsc_tiles[b] = expsc
                nc.scalar.activation(expsc[:], score_tiles.pop(b)[:], AF.Exp)

            def stage_zcol(b):
                zr_ps = ps_z.tile([128, 16], F32, tag="zr")
                zr_tiles[b] = zr_ps
                nc.tensor.matmul(
                    zr_ps[0:1, 0:NC_], ones_col[:], expsc_tiles[b][:],
                    start=True, stop=True,
                )

            def stage_softmax_b(b):
                """reciprocal + broadcast + alignment scale (DVE/PE-tiny)."""
                zr_ps = zr_tiles.pop(b)
                expsc = expsc_tiles.pop(b)
                zsum = wrk.tile([1, 1], F32, tag="zsum")
                nc.vector.reduce_sum(
                    zsum[:], zr_ps[0:1, 0:NC_], axis=mybir.AxisListType.X
                )
                rz = wrk.tile([1, 1], F32, tag="rz")
                nc.vector.reciprocal(rz[:], zsum[:])
                nc.tensor.matmul(
                    zr_ps[:, 8:9], ones_row[:], rz[:], start=True, stop=True
                )
                al_b = al_f32[:, b * NC_:(b + 1) * NC_]
                nc.vector.tensor_scalar_mul(al_b, expsc[:], zr_ps[:, 8:9])
                al_bf = wrk.tile([128, NC_], BF16, tag="al_bf")
                nc.vector.tensor_copy(al_bf[:], al_b)
                albf_tiles[b] = al_bf

            def stage_context(b, ps_c):
                al_bf = albf_tiles.pop(b)
                encn = encn_tiles.pop(b)
                c_ps = ps_c.tile([1, H], F32, tag="ctx")
                for k in range(NC_):
                    nc.tensor.matmul(
                        c_ps[:], al_bf[:, k:k + 1], encn[:, k],
                        start=(k == 0), stop=(k == NC_ - 1),
                    )
                nc.scalar.copy(ctx_sb[:, b * H:(b + 1) * H], c_ps[:])

            # 4-deep software pipeline: S(i) | v(i-1) | softmax(i-2) | ctx(i-3)
            with tc.tile_pool(name="ps_c", bufs=2, space="PSUM") as ps_c:
                for i in range(BL + 4):
                    if 2 <= i - 2 + 2 and 0 <= i - 2 < BL:
                        stage_exp(i - 2)        # ACT: first thing each round
                    if i < BL:
                        if i + 2 < BL:
                            load_enct(i + 2)
                        stage_s(i)
                    if 1 <= i - 2 < BL:
                        load_encn(i - 2)        # ctx(i-2) runs next iteration
                    if 0 <= i - 1 < BL:
                        stage_v(i - 1)
                    if 0 <= i - 2 < BL:
                        stage_zcol(i - 2)
                    if 0 <= i - 3 < BL:
                        stage_context(i - 3, ps_c)
                    if 0 <= i - 2 < BL:
                        stage_softmax_b(i - 2)

            # --- outputs ---
            # transpose alignment on PE so the store is one contiguous DMA:
            # al_f32 [p, (b tc)] -> al_t [(b tc), p]; then partition (b*8+tc)
            # holds t-run [tc*128 .. tc*128+128) of batch b -> flat [BL*T].
            with tc.tile_pool(name="ps_t", bufs=1, space="PSUM") as ps_t:
                al_tp = ps_t.tile([128, 128], F32)
                nc.tensor.transpose(al_tp[:], al_f32[:], ident[:])
                al_t = cp.tile([128, 128], F32)
                nc.vector.tensor_copy(al_t[:], al_tp[:])
            nc.sync.dma_start(
                al_out.rearrange("b t -> (b t)").rearrange("(q p) -> q p", p=128),
                al_t[:],
            )
            nc.sync.dma_start(ctx_out.rearrange("b h -> (b h)").unsqueeze(0), ctx_sb[:])

    nc.compile()
    return nc


_PROGRAM_CACHE: list = []


def _get_program() -> bacc.Bacc:
    if not _PROGRAM_CACHE:
        _PROGRAM_CACHE.append(_build_program())
    return _PROGRAM_CACHE[0]


def _host_shard(encoded_tokens, query, previous_alignment, conv_filter, W_loc, W_q, W_k, v):
    bf16 = ml_dtypes.bfloat16
    enc_bt = np.ascontiguousarray(encoded_tokens.transpose(1, 0, 2))  # [B, T, H]
    encN_all = enc_bt.astype(bf16)                                    # [B, T, H]
    encT_all = np.ascontiguousarray(enc_bt.transpose(0, 2, 1)).astype(bf16)  # [B, H, T]

    # device layout [p, (hk x)]: row p holds W[hk*128+p, :] for hk = 0..3
    def chunked(w, dt):
        return np.ascontiguousarray(
            w.reshape(HK, 128, -1).transpose(1, 0, 2).reshape(128, -1)
        ).astype(dt)

    wk_h = chunked(W_k, bf16)
    wq_h = chunked(W_q, np.float32)
    v_h = v.reshape(A, 1).astype(bf16)
    filt_h = np.ascontiguousarray(conv_filter.reshape(K, F)).astype(np.float32)
    wloc_h = W_loc.astype(np.float32)

    in_maps = []
    for c in range(NCORES):
        b0 = c * BL
        pa_pad = np.zeros((BL, ROW), np.float32)
        pa_pad[:, PAD:PAD + T] = previous_alignment[:, b0:b0 + BL].T
        pa_flat = np.zeros(PA_LEN, np.float32)
        pa_flat[:BL * ROW] = pa_pad.reshape(-1)
        in_maps.append({
            "encT": encT_all[b0:b0 + BL],
            "encN": encN_all[b0:b0 + BL],
            "paT": pa_flat.astype(bf16),
            "qT": chunked(query[b0:b0 + BL].T, np.float32),
            "wk": wk_h,
            "wq": wq_h,
            "wloc": wloc_h,
            "filt": filt_h,
            "vv": v_h,
        })
    return in_maps


def run(inputs: dict, trace: bool = False):
    nc = _get_program()
    in_maps = _host_shard(**inputs)
    res = run_bass_kernel_spmd(nc, in_maps, list(range(NCORES)), trace=trace)
    context = np.concatenate([res.results[c]["ctx_out"] for c in range(NCORES)], axis=0)
    # al_out is [BL, T] per core -> [B, T] -> transpose to [T, B]
    al_bt = np.concatenate([res.results[c]["al_out"] for c in range(NCORES)], axis=0)
    alignment = np.ascontiguousarray(al_bt.T)
    return (context.astype(np.float32), alignment.astype(np.float32)), res


def kernel(**inputs):
    (context, alignment), _ = run(inputs, trace=False)
    return context, alignment
